# revision 2
# baseline (speedup 1.0000x reference)
"""MoE layer (8 experts, top-2) on 8 TRN2 NeuronCores, expert-parallel.

Host: router matmul + top-2 + softmax + dispatch (gather tokens per expert,
bf16-cast, transpose) and the final weighted combine (scatter-add).
Device (SPMD, core e == expert e): the three 1024x1024 FFN matmuls + SiLU
in bf16 with f32 PSUM accumulation, token-transposed layout so the
contraction dim lives on SBUF partitions.
"""

import numpy as np
import ml_dtypes

import concourse.bass as bass
from concourse import bacc
import concourse.mybir as mybir
from concourse.tile import TileContext
from concourse.bass_utils import run_bass_kernel_spmd

EMBED = 1024
NUM_EXPERTS = 8
TOP_K = 2
P = 128
KCH = EMBED // P  # 8 contraction chunks of 128
TOK_TILE = 512

_nc_cache: dict[int, object] = {}
RUN_KWARGS: dict = {}
LAST_RESULT = None
LAST_EXEC_TIME_NS = None


def _build(cap: int):
    """Bass program for one core: out[1024, cap] = FFN_expert(xt[1024, cap])."""
    nc = bacc.Bacc()
    bf16 = mybir.dt.bfloat16
    f32 = mybir.dt.float32

    xt = nc.declare_dram_parameter("xt", [EMBED, cap], bf16, isOutput=False)
    wg = nc.declare_dram_parameter("wg", [EMBED, EMBED], bf16, isOutput=False)
    wu = nc.declare_dram_parameter("wu", [EMBED, EMBED], bf16, isOutput=False)
    wd = nc.declare_dram_parameter("wd", [EMBED, EMBED], bf16, isOutput=False)
    out = nc.declare_dram_parameter("out", [EMBED, cap], f32, isOutput=True)

    xt_r = xt.rearrange("(ko p) c -> p ko c", p=P)
    wg_r = wg.rearrange("(ko p) n -> p ko n", p=P)
    wu_r = wu.rearrange("(ko p) n -> p ko n", p=P)
    wd_r = wd.rearrange("(ko p) n -> p ko n", p=P)
    out_r = out.rearrange("(jo p) c -> p jo c", p=P)

    ntiles = cap // TOK_TILE

    with TileContext(nc) as tc:
        with (
            tc.tile_pool(name="wpool", bufs=1) as wpool,
            tc.tile_pool(name="xpool", bufs=3) as xpool,
            tc.tile_pool(name="apool", bufs=2) as apool,
            tc.tile_pool(name="spool", bufs=3) as spool,
            tc.tile_pool(name="opool", bufs=3) as opool,
            tc.tile_pool(name="psgu", bufs=2, space="PSUM") as psgu,
            tc.tile_pool(name="psd", bufs=2, space="PSUM") as psd,
        ):
            wg_sb = wpool.tile([P, KCH, EMBED], bf16, tag="wg")
            wu_sb = wpool.tile([P, KCH, EMBED], bf16, tag="wu")
            wd_sb = wpool.tile([P, KCH, EMBED], bf16, tag="wd")
            nc.sync.dma_start(wg_sb[:], wg_r)
            nc.sync.dma_start(wu_sb[:], wu_r)
            nc.sync.dma_start(wd_sb[:], wd_r)

            for t in range(ntiles):
                tok = slice(t * TOK_TILE, (t + 1) * TOK_TILE)
                xt_sb = xpool.tile([P, KCH, TOK_TILE], bf16, tag="xt")
                nc.sync.dma_start(xt_sb[:], xt_r[:, :, tok])

                act_sb = apool.tile([P, KCH, TOK_TILE], bf16, tag="act")
                for j in range(KCH):
                    jsl = slice(j * P, (j + 1) * P)
                    ps_g = psgu.tile([P, TOK_TILE], f32, tag="ps_gu")
                    ps_u = psgu.tile([P, TOK_TILE], f32, tag="ps_gu")
                    for k in range(KCH):
                        nc.tensor.matmul(
                            ps_g, wg_sb[:, k, jsl], xt_sb[:, k, :],
                            start=(k == 0), stop=(k == KCH - 1),
                        )
                    for k in range(KCH):
                        nc.tensor.matmul(
                            ps_u, wu_sb[:, k, jsl], xt_sb[:, k, :],
                            start=(k == 0), stop=(k == KCH - 1),
                        )
                    silu_sb = spool.tile([P, TOK_TILE], f32, tag="silu")
                    nc.scalar.activation(
                        silu_sb[:], ps_g[:], mybir.ActivationFunctionType.Silu
                    )
                    nc.vector.tensor_tensor(
                        act_sb[:, j, :], ps_u[:], silu_sb[:], mybir.AluOpType.mult
                    )

                for j in range(KCH):
                    jsl = slice(j * P, (j + 1) * P)
                    ps_d = psd.tile([P, TOK_TILE], f32, tag="ps_d")
                    for k in range(KCH):
                        nc.tensor.matmul(
                            ps_d, wd_sb[:, k, jsl], act_sb[:, k, :],
                            start=(k == 0), stop=(k == KCH - 1),
                        )
                    o_sb = opool.tile([P, TOK_TILE], f32, tag="o")
                    nc.vector.tensor_copy(o_sb[:], ps_d[:])
                    nc.sync.dma_start(out_r[:, j, tok], o_sb[:])

    nc.finalize()
    return nc


def kernel(x, router_w, router_b, we_gate, we_up, we_down):
    x = np.asarray(x, np.float32)
    router_w = np.asarray(router_w, np.float32)
    router_b = np.asarray(router_b, np.float32)
    we_gate = np.asarray(we_gate, np.float32)
    we_up = np.asarray(we_up, np.float32)
    we_down = np.asarray(we_down, np.float32)

    x_shape = x.shape
    x_flat = x.reshape(-1, x_shape[-1])
    T = x_flat.shape[0]

    # ---- host router (this decides the sharding) ----
    logits = x_flat @ router_w + router_b  # [T, E]
    topk_idx = np.argsort(logits, axis=-1)[:, ::-1][:, :TOP_K]  # [T, K] desc
    topk_val = np.take_along_axis(logits, topk_idx, axis=-1)
    m = topk_val.max(axis=-1, keepdims=True)
    e = np.exp(topk_val - m)
    topk_w = e / e.sum(axis=-1, keepdims=True)  # [T, K]

    # per-expert token lists
    tok_idx = [None] * NUM_EXPERTS
    tok_w = [None] * NUM_EXPERTS
    for ex in range(NUM_EXPERTS):
        rows, cols = np.nonzero(topk_idx == ex)
        tok_idx[ex] = rows
        tok_w[ex] = topk_w[rows, cols]
    counts = np.array([len(t) for t in tok_idx])
    cap = int(-(-counts.max() // TOK_TILE) * TOK_TILE)

    nc = _nc_cache.get(cap)
    if nc is None:
        nc = _nc_cache[cap] = _build(cap)

    # ---- dispatch: gather + bf16 cast + transpose per expert ----
    in_maps = []
    for ex in range(NUM_EXPERTS):
        xt = np.zeros((EMBED, cap), ml_dtypes.bfloat16)
        cnt = counts[ex]
        xt[:, :cnt] = x_flat[tok_idx[ex]].T.astype(ml_dtypes.bfloat16)
        in_maps.append(
            {
                "xt": xt,
                "wg": we_gate[ex].astype(ml_dtypes.bfloat16),
                "wu": we_up[ex].astype(ml_dtypes.bfloat16),
                "wd": we_down[ex].astype(ml_dtypes.bfloat16),
            }
        )

    global LAST_RESULT
    LAST_RESULT = run_bass_kernel_spmd(
        nc, in_maps, list(range(NUM_EXPERTS)), **RUN_KWARGS
    )
    global LAST_EXEC_TIME_NS
    LAST_EXEC_TIME_NS = LAST_RESULT.exec_time_ns
    res = LAST_RESULT.results

    # ---- combine: weighted scatter-add (token appears once per expert) ----
    out_flat = np.zeros_like(x_flat)
    for ex in range(NUM_EXPERTS):
        cnt = counts[ex]
        if cnt == 0:
            continue
        contrib = res[ex]["out"][:, :cnt].T * tok_w[ex][:, None].astype(np.float32)
        out_flat[tok_idx[ex]] += contrib
    return out_flat.reshape(x_shape)


# revision 5
# speedup vs baseline: 1.1247x; 1.1247x over previous
"""MoE layer (8 experts, top-2) on 8 TRN2 NeuronCores, expert-parallel.

Host: router matmul + top-2 + softmax + dispatch (gather tokens per expert,
bf16-cast, transpose) and the final weighted combine (scatter-add).
Device (SPMD, core e == expert e): the three 1024x1024 FFN matmuls + SiLU
in bf16 with f32 PSUM accumulation, token-transposed layout so the
contraction dim lives on SBUF partitions.
"""

import numpy as np
import ml_dtypes

import concourse.bass as bass
from concourse import bacc
import concourse.mybir as mybir
from concourse.tile import TileContext
from concourse.bass_utils import run_bass_kernel_spmd

EMBED = 1024
NUM_EXPERTS = 8
TOP_K = 2
P = 128
KCH = EMBED // P  # 8 contraction chunks of 128
TOK_TILE = 512

_nc_cache: dict[int, object] = {}
RUN_KWARGS: dict = {}
LAST_RESULT = None
LAST_EXEC_TIME_NS = None


def _build(cap: int):
    """Bass program for one core: out[1024, cap] = FFN_expert(xt[1024, cap])."""
    nc = bacc.Bacc()
    bf16 = mybir.dt.bfloat16
    f32 = mybir.dt.float32

    xt = nc.declare_dram_parameter("xt", [EMBED, cap], bf16, isOutput=False)
    wg = nc.declare_dram_parameter("wg", [EMBED, EMBED], bf16, isOutput=False)
    wu = nc.declare_dram_parameter("wu", [EMBED, EMBED], bf16, isOutput=False)
    wd = nc.declare_dram_parameter("wd", [EMBED, EMBED], bf16, isOutput=False)
    out = nc.declare_dram_parameter("out", [EMBED, cap], f32, isOutput=True)

    xt_r = xt.rearrange("(ko p) c -> p ko c", p=P)
    wg_r = wg.rearrange("(ko p) n -> p ko n", p=P)
    wu_r = wu.rearrange("(ko p) n -> p ko n", p=P)
    wd_r = wd.rearrange("(ko p) n -> p ko n", p=P)
    out_r = out.rearrange("(jo p) c -> p jo c", p=P)

    # tile widths: 512s plus one fine-grained remainder (cap % 512, multiple of 128)
    widths = [TOK_TILE] * (cap // TOK_TILE)
    if cap % TOK_TILE:
        widths.append(cap % TOK_TILE)

    with TileContext(nc) as tc:
        with (
            tc.tile_pool(name="wpool", bufs=1) as wpool,
            tc.tile_pool(name="xpool", bufs=3) as xpool,
            tc.tile_pool(name="apool", bufs=2) as apool,
            tc.tile_pool(name="spool", bufs=3) as spool,
            tc.tile_pool(name="opool", bufs=3) as opool,
            tc.tile_pool(name="psgu", bufs=2, space="PSUM") as psgu,
            tc.tile_pool(name="psd", bufs=2, space="PSUM") as psd,
        ):
            wg_sb = wpool.tile([P, KCH, EMBED], bf16, tag="wg")
            wu_sb = wpool.tile([P, KCH, EMBED], bf16, tag="wu")
            wd_sb = wpool.tile([P, KCH, EMBED], bf16, tag="wd")
            # tile-0 tokens, chunk-interleaved with wg so the PE starts ~1us in
            # (it needs only wg[k] + xt0[k] for the k-th gate accumulation step)
            xt0_sb = xpool.tile([P, KCH, TOK_TILE], bf16, tag="xt")
            for k in range(KCH):
                nc.sync.dma_start(wg_sb[:, k, :], wg_r[:, k, :])
                nc.sync.dma_start(xt0_sb[:, k, :], xt_r[:, k, :TOK_TILE])
            nc.sync.dma_start(wu_sb[:], wu_r)
            nc.sync.dma_start(wd_sb[:], wd_r)

            pos = 0
            for t, w in enumerate(widths):
                tok = slice(pos, pos + w)
                if t == 0:
                    xt_sb = xt0_sb
                else:
                    xt_sb = xpool.tile([P, KCH, TOK_TILE], bf16, tag="xt")
                    nc.sync.dma_start(xt_sb[:, :, :w], xt_r[:, :, tok])

                act_sb = apool.tile([P, KCH, TOK_TILE], bf16, tag="act")
                for j in range(KCH):
                    jsl = slice(j * P, (j + 1) * P)
                    ps_g = psgu.tile([P, TOK_TILE], f32, tag="ps_gu", name="ps_g")[:, :w]
                    ps_u = psgu.tile([P, TOK_TILE], f32, tag="ps_gu", name="ps_u")[:, :w]
                    for k in range(KCH):
                        nc.tensor.matmul(
                            ps_g, wg_sb[:, k, jsl], xt_sb[:, k, :w],
                            start=(k == 0), stop=(k == KCH - 1),
                        )
                    for k in range(KCH):
                        nc.tensor.matmul(
                            ps_u, wu_sb[:, k, jsl], xt_sb[:, k, :w],
                            start=(k == 0), stop=(k == KCH - 1),
                        )
                    silu_sb = spool.tile([P, TOK_TILE], f32, tag="silu", name="silu")[:, :w]
                    nc.scalar.activation(
                        silu_sb, ps_g, mybir.ActivationFunctionType.Silu
                    )
                    nc.vector.tensor_tensor(
                        act_sb[:, j, :w], ps_u, silu_sb, mybir.AluOpType.mult
                    )

                for j in range(KCH):
                    jsl = slice(j * P, (j + 1) * P)
                    ps_d = psd.tile([P, TOK_TILE], f32, tag="ps_d", name="ps_d")[:, :w]
                    for k in range(KCH):
                        nc.tensor.matmul(
                            ps_d, wd_sb[:, k, jsl], act_sb[:, k, :w],
                            start=(k == 0), stop=(k == KCH - 1),
                        )
                    o_sb = opool.tile([P, TOK_TILE], f32, tag="o", name="o")[:, :w]
                    nc.vector.tensor_copy(o_sb, ps_d)
                    nc.sync.dma_start(out_r[:, j, tok], o_sb)
                pos += w

    nc.finalize()
    return nc


def kernel(x, router_w, router_b, we_gate, we_up, we_down):
    x = np.asarray(x, np.float32)
    router_w = np.asarray(router_w, np.float32)
    router_b = np.asarray(router_b, np.float32)
    we_gate = np.asarray(we_gate, np.float32)
    we_up = np.asarray(we_up, np.float32)
    we_down = np.asarray(we_down, np.float32)

    x_shape = x.shape
    x_flat = x.reshape(-1, x_shape[-1])
    T = x_flat.shape[0]

    # ---- host router (this decides the sharding) ----
    logits = x_flat @ router_w + router_b  # [T, E]
    topk_idx = np.argsort(logits, axis=-1)[:, ::-1][:, :TOP_K]  # [T, K] desc
    topk_val = np.take_along_axis(logits, topk_idx, axis=-1)
    m = topk_val.max(axis=-1, keepdims=True)
    e = np.exp(topk_val - m)
    topk_w = e / e.sum(axis=-1, keepdims=True)  # [T, K]

    # per-expert token lists
    tok_idx = [None] * NUM_EXPERTS
    tok_w = [None] * NUM_EXPERTS
    for ex in range(NUM_EXPERTS):
        rows, cols = np.nonzero(topk_idx == ex)
        tok_idx[ex] = rows
        tok_w[ex] = topk_w[rows, cols]
    counts = np.array([len(t) for t in tok_idx])
    cap = int(-(-counts.max() // P) * P)  # 128-granular capacity

    nc = _nc_cache.get(cap)
    if nc is None:
        nc = _nc_cache[cap] = _build(cap)

    # ---- dispatch: gather + bf16 cast + transpose per expert ----
    in_maps = []
    for ex in range(NUM_EXPERTS):
        xt = np.zeros((EMBED, cap), ml_dtypes.bfloat16)
        cnt = counts[ex]
        xt[:, :cnt] = x_flat[tok_idx[ex]].T.astype(ml_dtypes.bfloat16)
        in_maps.append(
            {
                "xt": xt,
                "wg": we_gate[ex].astype(ml_dtypes.bfloat16),
                "wu": we_up[ex].astype(ml_dtypes.bfloat16),
                "wd": we_down[ex].astype(ml_dtypes.bfloat16),
            }
        )

    global LAST_RESULT
    LAST_RESULT = run_bass_kernel_spmd(
        nc, in_maps, list(range(NUM_EXPERTS)), **RUN_KWARGS
    )
    global LAST_EXEC_TIME_NS
    LAST_EXEC_TIME_NS = LAST_RESULT.exec_time_ns
    res = LAST_RESULT.results

    # ---- combine: weighted scatter-add (token appears once per expert) ----
    out_flat = np.zeros_like(x_flat)
    for ex in range(NUM_EXPERTS):
        cnt = counts[ex]
        if cnt == 0:
            continue
        contrib = res[ex]["out"][:, :cnt].T * tok_w[ex][:, None].astype(np.float32)
        out_flat[tok_idx[ex]] += contrib
    return out_flat.reshape(x_shape)


# revision 7
# speedup vs baseline: 1.1629x; 1.0340x over previous
"""MoE layer (8 experts, top-2) on 8 TRN2 NeuronCores, expert-parallel.

Host: router matmul + top-2 + softmax + dispatch (gather tokens per expert,
bf16-cast, transpose) and the final weighted combine (scatter-add).
Device (SPMD, core e == expert e): the three 1024x1024 FFN matmuls + SiLU
in bf16 with f32 PSUM accumulation, token-transposed layout so the
contraction dim lives on SBUF partitions.
"""

import numpy as np
import ml_dtypes

import concourse.bass as bass
from concourse import bacc
import concourse.mybir as mybir
from concourse.tile import TileContext
from concourse.bass_utils import run_bass_kernel_spmd

EMBED = 1024
NUM_EXPERTS = 8
TOP_K = 2
P = 128
KCH = EMBED // P  # 8 contraction chunks of 128
TOK_TILE = 512

_nc_cache: dict[int, object] = {}
RUN_KWARGS: dict = {}
LAST_RESULT = None
LAST_EXEC_TIME_NS = None


def _build(cap: int):
    """Bass program for one core: out[1024, cap] = FFN_expert(xt[1024, cap])."""
    nc = bacc.Bacc()
    bf16 = mybir.dt.bfloat16
    f32 = mybir.dt.float32

    xt = nc.declare_dram_parameter("xt", [EMBED, cap], bf16, isOutput=False)
    wg = nc.declare_dram_parameter("wg", [EMBED, EMBED], bf16, isOutput=False)
    wu = nc.declare_dram_parameter("wu", [EMBED, EMBED], bf16, isOutput=False)
    wd = nc.declare_dram_parameter("wd", [EMBED, EMBED], bf16, isOutput=False)
    out = nc.declare_dram_parameter("out", [EMBED, cap], f32, isOutput=True)

    xt_r = xt.rearrange("(ko p) c -> p ko c", p=P)
    wg_r = wg.rearrange("(ko p) n -> p ko n", p=P)
    wu_r = wu.rearrange("(ko p) n -> p ko n", p=P)
    wd_r = wd.rearrange("(ko p) n -> p ko n", p=P)
    out_r = out.rearrange("(jo p) c -> p jo c", p=P)

    # tile widths: 512s plus one fine-grained remainder (cap % 512, multiple of 128)
    widths = [TOK_TILE] * (cap // TOK_TILE)
    if cap % TOK_TILE:
        widths.append(cap % TOK_TILE)

    with TileContext(nc) as tc:
        with (
            tc.tile_pool(name="wpool", bufs=1) as wpool,
            tc.tile_pool(name="xpool", bufs=3) as xpool,
            tc.tile_pool(name="apool", bufs=2) as apool,
            tc.tile_pool(name="spool", bufs=2) as spool,
            tc.tile_pool(name="opool", bufs=3) as opool,
            tc.tile_pool(name="psgu", bufs=2, space="PSUM") as psgu,
            tc.tile_pool(name="psd", bufs=2, space="PSUM") as psd,
        ):
            wg_sb = wpool.tile([P, KCH, EMBED], bf16, tag="wg")
            wu_sb = wpool.tile([P, KCH, EMBED], bf16, tag="wu")
            wd_sb = wpool.tile([P, KCH, EMBED], bf16, tag="wd")
            # tile-0 tokens, chunk-interleaved with wg so the PE starts ~1us in
            # (it needs only wg[k] + xt0[k] for the k-th gate accumulation step)
            xt0_sb = xpool.tile([P, KCH, TOK_TILE], bf16, tag="xt")
            for k in range(KCH):
                nc.sync.dma_start(wg_sb[:, k, :], wg_r[:, k, :])
                nc.sync.dma_start(xt0_sb[:, k, :], xt_r[:, k, :TOK_TILE])
            nc.sync.dma_start(wu_sb[:], wu_r)
            nc.sync.dma_start(wd_sb[:], wd_r)

            pos = 0
            for t, w in enumerate(widths):
                tok = slice(pos, pos + w)
                if t == 0:
                    xt_sb = xt0_sb
                else:
                    xt_sb = xpool.tile([P, KCH, TOK_TILE], bf16, tag="xt")
                    nc.sync.dma_start(xt_sb[:, :, :w], xt_r[:, :, tok])

                act_sb = apool.tile([P, KCH, TOK_TILE], bf16, tag="act")
                silu_sb = spool.tile([P, KCH, TOK_TILE], f32, tag="silu")
                # all gate matmuls first: they only need wg, so the PE ramps
                # at t~1us while wu/wd are still streaming in
                for j in range(KCH):
                    jsl = slice(j * P, (j + 1) * P)
                    ps_g = psgu.tile([P, TOK_TILE], f32, tag="ps_gu", name="ps_g")[:, :w]
                    for k in range(KCH):
                        nc.tensor.matmul(
                            ps_g, wg_sb[:, k, jsl], xt_sb[:, k, :w],
                            start=(k == 0), stop=(k == KCH - 1),
                        )
                    nc.scalar.activation(
                        silu_sb[:, j, :w], ps_g, mybir.ActivationFunctionType.Silu
                    )
                for j in range(KCH):
                    jsl = slice(j * P, (j + 1) * P)
                    ps_u = psgu.tile([P, TOK_TILE], f32, tag="ps_gu", name="ps_u")[:, :w]
                    for k in range(KCH):
                        nc.tensor.matmul(
                            ps_u, wu_sb[:, k, jsl], xt_sb[:, k, :w],
                            start=(k == 0), stop=(k == KCH - 1),
                        )
                    nc.vector.tensor_tensor(
                        act_sb[:, j, :w], ps_u, silu_sb[:, j, :w], mybir.AluOpType.mult
                    )

                for j in range(KCH):
                    jsl = slice(j * P, (j + 1) * P)
                    ps_d = psd.tile([P, TOK_TILE], f32, tag="ps_d", name="ps_d")[:, :w]
                    for k in range(KCH):
                        nc.tensor.matmul(
                            ps_d, wd_sb[:, k, jsl], act_sb[:, k, :w],
                            start=(k == 0), stop=(k == KCH - 1),
                        )
                    o_sb = opool.tile([P, TOK_TILE], f32, tag="o", name="o")[:, :w]
                    nc.vector.tensor_copy(o_sb, ps_d)
                    nc.sync.dma_start(out_r[:, j, tok], o_sb)
                pos += w

    nc.finalize()
    return nc


def kernel(x, router_w, router_b, we_gate, we_up, we_down):
    x = np.asarray(x, np.float32)
    router_w = np.asarray(router_w, np.float32)
    router_b = np.asarray(router_b, np.float32)
    we_gate = np.asarray(we_gate, np.float32)
    we_up = np.asarray(we_up, np.float32)
    we_down = np.asarray(we_down, np.float32)

    x_shape = x.shape
    x_flat = x.reshape(-1, x_shape[-1])
    T = x_flat.shape[0]

    # ---- host router (this decides the sharding) ----
    logits = x_flat @ router_w + router_b  # [T, E]
    topk_idx = np.argsort(logits, axis=-1)[:, ::-1][:, :TOP_K]  # [T, K] desc
    topk_val = np.take_along_axis(logits, topk_idx, axis=-1)
    m = topk_val.max(axis=-1, keepdims=True)
    e = np.exp(topk_val - m)
    topk_w = e / e.sum(axis=-1, keepdims=True)  # [T, K]

    # per-expert token lists
    tok_idx = [None] * NUM_EXPERTS
    tok_w = [None] * NUM_EXPERTS
    for ex in range(NUM_EXPERTS):
        rows, cols = np.nonzero(topk_idx == ex)
        tok_idx[ex] = rows
        tok_w[ex] = topk_w[rows, cols]
    counts = np.array([len(t) for t in tok_idx])
    cap = int(-(-counts.max() // P) * P)  # 128-granular capacity

    nc = _nc_cache.get(cap)
    if nc is None:
        nc = _nc_cache[cap] = _build(cap)

    # ---- dispatch: gather + bf16 cast + transpose per expert ----
    in_maps = []
    for ex in range(NUM_EXPERTS):
        xt = np.zeros((EMBED, cap), ml_dtypes.bfloat16)
        cnt = counts[ex]
        xt[:, :cnt] = x_flat[tok_idx[ex]].T.astype(ml_dtypes.bfloat16)
        in_maps.append(
            {
                "xt": xt,
                "wg": we_gate[ex].astype(ml_dtypes.bfloat16),
                "wu": we_up[ex].astype(ml_dtypes.bfloat16),
                "wd": we_down[ex].astype(ml_dtypes.bfloat16),
            }
        )

    global LAST_RESULT
    LAST_RESULT = run_bass_kernel_spmd(
        nc, in_maps, list(range(NUM_EXPERTS)), **RUN_KWARGS
    )
    global LAST_EXEC_TIME_NS
    LAST_EXEC_TIME_NS = LAST_RESULT.exec_time_ns
    res = LAST_RESULT.results

    # ---- combine: weighted scatter-add (token appears once per expert) ----
    out_flat = np.zeros_like(x_flat)
    for ex in range(NUM_EXPERTS):
        cnt = counts[ex]
        if cnt == 0:
            continue
        contrib = res[ex]["out"][:, :cnt].T * tok_w[ex][:, None].astype(np.float32)
        out_flat[tok_idx[ex]] += contrib
    return out_flat.reshape(x_shape)


# revision 8
# speedup vs baseline: 1.1759x; 1.0112x over previous
"""MoE layer (8 experts, top-2) on 8 TRN2 NeuronCores, expert-parallel.

Host: router matmul + top-2 + softmax + dispatch (gather tokens per expert,
bf16-cast, transpose) and the final weighted combine (scatter-add).
Device (SPMD, core e == expert e): the three 1024x1024 FFN matmuls + SiLU
in bf16 with f32 PSUM accumulation, token-transposed layout so the
contraction dim lives on SBUF partitions.
"""

import numpy as np
import ml_dtypes

import concourse.bass as bass
from concourse import bacc
import concourse.mybir as mybir
from concourse.tile import TileContext
from concourse.bass_utils import run_bass_kernel_spmd

EMBED = 1024
NUM_EXPERTS = 8
TOP_K = 2
P = 128
KCH = EMBED // P  # 8 contraction chunks of 128
TOK_TILE = 512

_nc_cache: dict[int, object] = {}
RUN_KWARGS: dict = {}
LAST_RESULT = None
LAST_EXEC_TIME_NS = None


def _build(cap: int):
    """Bass program for one core: out[1024, cap] = FFN_expert(xt[1024, cap])."""
    nc = bacc.Bacc()
    bf16 = mybir.dt.bfloat16
    f32 = mybir.dt.float32

    xt = nc.declare_dram_parameter("xt", [EMBED, cap], bf16, isOutput=False)
    wg = nc.declare_dram_parameter("wg", [EMBED, EMBED], bf16, isOutput=False)
    wu = nc.declare_dram_parameter("wu", [EMBED, EMBED], bf16, isOutput=False)
    wd = nc.declare_dram_parameter("wd", [EMBED, EMBED], bf16, isOutput=False)
    out = nc.declare_dram_parameter("out", [EMBED, cap], f32, isOutput=True)

    xt_r = xt.rearrange("(ko p) c -> p ko c", p=P)
    wg_r = wg.rearrange("(ko p) n -> p ko n", p=P)
    wu_r = wu.rearrange("(ko p) n -> p ko n", p=P)
    wd_r = wd.rearrange("(ko p) n -> p ko n", p=P)
    out_r = out.rearrange("(jo p) c -> p jo c", p=P)

    # tile widths: 512s plus one fine-grained remainder (cap % 512, multiple of 128)
    widths = [TOK_TILE] * (cap // TOK_TILE)
    if cap % TOK_TILE:
        widths.append(cap % TOK_TILE)

    with TileContext(nc) as tc:
        with (
            tc.tile_pool(name="wpool", bufs=1) as wpool,
            tc.tile_pool(name="xpool", bufs=3) as xpool,
            tc.tile_pool(name="apool", bufs=2) as apool,
            tc.tile_pool(name="spool", bufs=2) as spool,
            tc.tile_pool(name="opool", bufs=3) as opool,
            tc.tile_pool(name="psgu", bufs=3, space="PSUM") as psgu,
            tc.tile_pool(name="psd", bufs=3, space="PSUM") as psd,
        ):
            wg_sb = wpool.tile([P, KCH, EMBED], bf16, tag="wg")
            wu_sb = wpool.tile([P, KCH, EMBED], bf16, tag="wu")
            wd_sb = wpool.tile([P, KCH, EMBED], bf16, tag="wd")
            # tile-0 tokens, chunk-interleaved with wg so the PE starts ~1us in
            # (it needs only wg[k] + xt0[k] for the k-th gate accumulation step)
            xt0_sb = xpool.tile([P, KCH, TOK_TILE], bf16, tag="xt")
            for k in range(KCH):
                nc.sync.dma_start(wg_sb[:, k, :], wg_r[:, k, :])
                nc.sync.dma_start(xt0_sb[:, k, :], xt_r[:, k, :TOK_TILE])
            nc.sync.dma_start(wu_sb[:], wu_r)
            nc.sync.dma_start(wd_sb[:], wd_r)

            pos = 0
            for t, w in enumerate(widths):
                tok = slice(pos, pos + w)
                if t == 0:
                    xt_sb = xt0_sb
                else:
                    xt_sb = xpool.tile([P, KCH, TOK_TILE], bf16, tag="xt")
                    nc.sync.dma_start(xt_sb[:, :, :w], xt_r[:, :, tok])

                act_sb = apool.tile([P, KCH, TOK_TILE], bf16, tag="act")
                silu_sb = spool.tile([P, KCH, TOK_TILE], f32, tag="silu")
                # all gate matmuls first: they only need wg, so the PE ramps
                # at t~1us while wu/wd are still streaming in
                for j in range(KCH):
                    jsl = slice(j * P, (j + 1) * P)
                    ps_g = psgu.tile([P, TOK_TILE], f32, tag="ps_gu", name="ps_g")[:, :w]
                    for k in range(KCH):
                        nc.tensor.matmul(
                            ps_g, wg_sb[:, k, jsl], xt_sb[:, k, :w],
                            start=(k == 0), stop=(k == KCH - 1),
                        )
                    nc.scalar.activation(
                        silu_sb[:, j, :w], ps_g, mybir.ActivationFunctionType.Silu
                    )
                for j in range(KCH):
                    jsl = slice(j * P, (j + 1) * P)
                    ps_u = psgu.tile([P, TOK_TILE], f32, tag="ps_gu", name="ps_u")[:, :w]
                    for k in range(KCH):
                        nc.tensor.matmul(
                            ps_u, wu_sb[:, k, jsl], xt_sb[:, k, :w],
                            start=(k == 0), stop=(k == KCH - 1),
                        )
                    nc.vector.tensor_tensor(
                        act_sb[:, j, :w], ps_u, silu_sb[:, j, :w], mybir.AluOpType.mult
                    )

                for j in range(KCH):
                    jsl = slice(j * P, (j + 1) * P)
                    ps_d = psd.tile([P, TOK_TILE], f32, tag="ps_d", name="ps_d")[:, :w]
                    for k in range(KCH):
                        nc.tensor.matmul(
                            ps_d, wd_sb[:, k, jsl], act_sb[:, k, :w],
                            start=(k == 0), stop=(k == KCH - 1),
                        )
                    o_sb = opool.tile([P, TOK_TILE], f32, tag="o", name="o")[:, :w]
                    nc.vector.tensor_copy(o_sb, ps_d)
                    nc.sync.dma_start(out_r[:, j, tok], o_sb)
                pos += w

    nc.finalize()
    return nc


def kernel(x, router_w, router_b, we_gate, we_up, we_down):
    x = np.asarray(x, np.float32)
    router_w = np.asarray(router_w, np.float32)
    router_b = np.asarray(router_b, np.float32)
    we_gate = np.asarray(we_gate, np.float32)
    we_up = np.asarray(we_up, np.float32)
    we_down = np.asarray(we_down, np.float32)

    x_shape = x.shape
    x_flat = x.reshape(-1, x_shape[-1])
    T = x_flat.shape[0]

    # ---- host router (this decides the sharding) ----
    logits = x_flat @ router_w + router_b  # [T, E]
    topk_idx = np.argsort(logits, axis=-1)[:, ::-1][:, :TOP_K]  # [T, K] desc
    topk_val = np.take_along_axis(logits, topk_idx, axis=-1)
    m = topk_val.max(axis=-1, keepdims=True)
    e = np.exp(topk_val - m)
    topk_w = e / e.sum(axis=-1, keepdims=True)  # [T, K]

    # per-expert token lists
    tok_idx = [None] * NUM_EXPERTS
    tok_w = [None] * NUM_EXPERTS
    for ex in range(NUM_EXPERTS):
        rows, cols = np.nonzero(topk_idx == ex)
        tok_idx[ex] = rows
        tok_w[ex] = topk_w[rows, cols]
    counts = np.array([len(t) for t in tok_idx])
    cap = int(-(-counts.max() // P) * P)  # 128-granular capacity

    nc = _nc_cache.get(cap)
    if nc is None:
        nc = _nc_cache[cap] = _build(cap)

    # ---- dispatch: gather + bf16 cast + transpose per expert ----
    in_maps = []
    for ex in range(NUM_EXPERTS):
        xt = np.zeros((EMBED, cap), ml_dtypes.bfloat16)
        cnt = counts[ex]
        xt[:, :cnt] = x_flat[tok_idx[ex]].T.astype(ml_dtypes.bfloat16)
        in_maps.append(
            {
                "xt": xt,
                "wg": we_gate[ex].astype(ml_dtypes.bfloat16),
                "wu": we_up[ex].astype(ml_dtypes.bfloat16),
                "wd": we_down[ex].astype(ml_dtypes.bfloat16),
            }
        )

    global LAST_RESULT
    LAST_RESULT = run_bass_kernel_spmd(
        nc, in_maps, list(range(NUM_EXPERTS)), **RUN_KWARGS
    )
    global LAST_EXEC_TIME_NS
    LAST_EXEC_TIME_NS = LAST_RESULT.exec_time_ns
    res = LAST_RESULT.results

    # ---- combine: weighted scatter-add (token appears once per expert) ----
    out_flat = np.zeros_like(x_flat)
    for ex in range(NUM_EXPERTS):
        cnt = counts[ex]
        if cnt == 0:
            continue
        contrib = res[ex]["out"][:, :cnt].T * tok_w[ex][:, None].astype(np.float32)
        out_flat[tok_idx[ex]] += contrib
    return out_flat.reshape(x_shape)
